# revision 17
# baseline (speedup 1.0000x reference)
"""Causal multi-head self-attention (B=2, S=2048, D=1024, H=16) on 8 trn2 cores.

Device kernel (unchanged from v1): tensor-parallel over heads — core c owns
heads (2c, 2c+1), both batches, full sequence. Per core: QKV projections for
its 2 heads, RoPE, causal attention, output-projection partial product into
yo [4096, 1024] bf16.

Dispatch layer (v2): the wall-clock metric is dominated by the ~50 MB/s
axon tunnel, so per-call host<->device traffic is cut from ~207 MB to
~16 MB up + 8 MB down:
  - x ships ONCE row-sharded (bf16, 1 MB/core); an XLA prep jit all-gathers
    it on device over NeuronLink, transposes to xt [D, R], and builds the
    RoPE cos/sin tables on device from token_positions (so the 12 MB of
    per-core tables never cross the tunnel).
  - per-core weight slices ship as one bf16 blob (8 MB total; wo is cast
    back to f32 on device).
  - no donated zero output buffers (v1 shipped 64 MB of zeros per call;
    yo is fully written by the kernel so uninit PJRT allocation is fine).
  - the 8 partial yo's are psum-reduced ON DEVICE (f32 accumulate) and a
    single bf16 [4096, 1024] replica is fetched (8 MB vs 64 MB).
  - all three jits (prep / bass_exec / reduce) are built once and cached;
    v1 re-traced + re-lowered (incl. zstd of the BIR) every call.
"""
import math
import numpy as np

import jax
import jax.numpy as jnp
from jax.sharding import Mesh, PartitionSpec as P, NamedSharding

from jax.experimental.shard_map import shard_map

import ml_dtypes

import concourse.bass as bass
from concourse import bacc
import concourse.mybir as mybir
from concourse.tile import TileContext
from concourse import bass2jax as b2j

THETA = 10000.0
B, S, D, H = 2, 2048, 1024, 16
DH = D // H          # 64
NC = 8               # cores
HPC = H // NC        # heads per core = 2
R = B * S            # 4096 flat rows
SCALE = 1.0 / math.sqrt(DH)

f32 = mybir.dt.float32
f32r = mybir.dt.float32r
bf16 = mybir.dt.bfloat16
npbf = ml_dtypes.bfloat16

_CACHE = {}


def _build(_DBG_REPS=1):
    nc = bacc.Bacc(num_devices=NC)

    xt = nc.declare_dram_parameter("xt", [D, R], bf16, isOutput=False)
    wq = nc.declare_dram_parameter("wq", [128, 8 * 2 * DH], bf16, isOutput=False)
    wk = nc.declare_dram_parameter("wk", [128, 8 * 2 * DH], bf16, isOutput=False)
    wv = nc.declare_dram_parameter("wv", [128, 8 * 2 * DH], bf16, isOutput=False)
    wo = nc.declare_dram_parameter("wo", [2 * DH, D], f32r, isOutput=False)
    cost = nc.declare_dram_parameter("cost", [128, S], f32, isOutput=False)
    sint = nc.declare_dram_parameter("sint", [128, S], bf16, isOutput=False)
    ones = nc.declare_dram_parameter("ones", [128, DH], f32r, isOutput=False)
    ident = nc.declare_dram_parameter("ident", [128, 128], f32, isOutput=False)
    trim = nc.declare_dram_parameter("trim", [128, 512], bf16, isOutput=False)
    yo = nc.declare_dram_parameter("yo", [R, D], bf16, isOutput=True)

    NQ = 4            # xT column quarters in phase A
    QW = R // NQ      # 1024 rows per quarter
    NB = QW // 512    # 2 proj psum blocks per quarter
    KT = S // 128     # 16 key tiles per batch

    with TileContext(nc) as tc:
        import contextlib
        ctx = contextlib.ExitStack()
        with ctx:
            # ---- persistent pools (whole kernel) ----
            pers = ctx.enter_context(tc.tile_pool(name="pers", bufs=1))
            exp_pool = ctx.enter_context(tc.tile_pool(name="expp", bufs=8))
            aux = ctx.enter_context(tc.tile_pool(name="aux", bufs=2))

            q_rope = pers.tile([128, R], bf16, name="q_rope")
            k_rope = pers.tile([128, R], bf16, name="k_rope")
            # V per batch, natural rows layout; per key tile 130 cols:
            #   per head: [v(64) | one]
            v_sb = [pers.tile([128, KT * 130], bf16, name=f"v_sb{b}") for b in range(B)]
            attn = [pers.tile([128, S], f32r, name=f"attn{b}") for b in range(B)]
            wo_sb = pers.tile([128, D], f32r, name="wo_sb")
            ones_sb = pers.tile([128, DH], f32r, name="ones_sb")
            id_sb = pers.tile([128, 128], f32, name="id_sb")
            tri_sb = pers.tile([128, 512], bf16, name="tri_sb")
            for b in range(B):
                nc.vector.memset(
                    v_sb[b][:, :].rearrange("p (t c) -> p t c", c=65)[:, :, 64:65], 1.0)

            # ---- phase A: projections + RoPE + V assembly ----
            for _rep in range(_DBG_REPS):
              with tc.tile_pool(name="phA", bufs=1) as pha, \
                   tc.tile_pool(name="xtp", bufs=2) as xtp, \
                   tc.tile_pool(name="ropetmp", bufs=2) as rtp, \
                   tc.tile_pool(name="prj_ps", bufs=6, space="PSUM") as prj, \
                   tc.tile_pool(name="tp_ps", bufs=2, space="PSUM") as tpp:

                  wq_sb = pha.tile([128, 8, 2 * DH], bf16, name="wq_sb")
                  wk_sb = pha.tile([128, 8, 2 * DH], bf16, name="wk_sb")
                  wv_sb = pha.tile([128, 8, 2 * DH], bf16, name="wv_sb")
                  cos_sb = pha.tile([128, S], f32, name="cos_sb")
                  sin_sb = pha.tile([128, S], bf16, name="sin_sb")
                  nc.sync.dma_start(out=wq_sb[:, :, :], in_=wq.rearrange("p (t m) -> p t m", t=8))
                  xtb0 = [xtp.tile([128, 4, QW], bf16, name=f"xt0{hh}", tag=f"xt{hh}")
                          for hh in range(2)]
                  for hh in range(2):
                      for sub in range(2):
                          nc.sync.dma_start(
                              out=xtb0[hh][:, sub * 2:(sub + 1) * 2, :],
                              in_=xt[hh * 512 + sub * 256:hh * 512 + (sub + 1) * 256,
                                     0:QW].rearrange("(t p) m -> p t m", p=128))
                  nc.sync.dma_start(out=wk_sb[:, :, :], in_=wk.rearrange("p (t m) -> p t m", t=8))
                  nc.sync.dma_start(out=wv_sb[:, :, :], in_=wv.rearrange("p (t m) -> p t m", t=8))
                  nc.sync.dma_start(out=cos_sb[:, :], in_=cost[:, :])
                  nc.sync.dma_start(out=sin_sb[:, :], in_=sint[:, :])
                  if _rep == 0:
                      nc.sync.dma_start(out=id_sb[:, :], in_=ident[:, :])
                      nc.sync.dma_start(out=tri_sb[:, :], in_=trim[:, :])
                      nc.sync.dma_start(out=ones_sb[:, :], in_=ones[:, :])
                      nc.sync.dma_start(out=wo_sb[:, :], in_=wo[:, :])

                  for qr in range(NQ):
                      c0 = qr * QW           # global row offset of this quarter
                      s0 = c0 % S            # seq offset of this quarter
                      if qr == 0:
                          xtb = xtb0
                      else:
                          xtb = xtb_next
                      if qr + 1 < NQ:
                          cn = (qr + 1) * QW
                          xtb_next = [xtp.tile([128, 4, QW], bf16, name=f"xt{qr + 1}{hh}", tag=f"xt{hh}")
                                      for hh in range(2)]
                          for hh in range(2):
                              nc.sync.dma_start(
                                  out=xtb_next[hh][:, :, :],
                                  in_=xt[hh * 512:(hh + 1) * 512, cn:cn + QW].rearrange(
                                      "(t p) m -> p t m", p=128))
                      xts = [xtb[k // 4][:, k % 4, :] for k in range(8)]

                      for tname, wsb, rope in (("q", wq_sb, q_rope), ("k", wk_sb, k_rope)):
                          pss = []
                          for n in range(NB):
                              ps = prj.tile([128, 512], f32, name=f"p{tname}{qr}{n}", tag="prj")
                              for k in range(8):
                                  nc.tensor.matmul(ps[:, :], wsb[:, k, :], xts[k][:, n * 512:(n + 1) * 512],
                                                   start=(k == 0), stop=(k == 7))
                              pss.append(ps)
                          # RoPE: per head rows [even(32) | odd(32)]; sin table
                          # carries the sign, so out = ps*cos + swap32(ps)*sin.
                          raw = rtp.tile([128, QW], bf16, name=f"raw{tname}", tag="raw")
                          pcs = rtp.tile([128, QW], bf16, name=f"pcs{tname}", tag="pcs")
                          for n in range(NB):
                              cs = slice(n * 512, (n + 1) * 512)
                              gs = slice(s0 + n * 512, s0 + (n + 1) * 512)
                              nc.vector.tensor_copy(raw[:, cs], pss[n][:, :])
                              nc.vector.tensor_tensor(pcs[:, cs], pss[n][:, :], cos_sb[:, gs], mybir.AluOpType.mult)
                          swp = rtp.tile([128, QW], bf16, name=f"swp{tname}", tag="swp")
                          for r0 in range(0, 128, 64):
                              nc.sync.dma_start(out=swp[r0:r0 + 32, :], in_=raw[r0 + 32:r0 + 64, :])
                              nc.sync.dma_start(out=swp[r0 + 32:r0 + 64, :], in_=raw[r0:r0 + 32, :])
                          gq = slice(s0, s0 + QW)
                          tsn = rtp.tile([128, QW], bf16, name=f"tsn{tname}", tag="tsn")
                          nc.vector.tensor_tensor(tsn[:, :], swp[:, :], sin_sb[:, gq], mybir.AluOpType.mult)
                          nc.vector.tensor_tensor(rope[:, c0:c0 + QW], pcs[:, :], tsn[:, :], mybir.AluOpType.add)

                      # V: transposed projection (N=512) then PE-transpose to natural
                      vt_sb = rtp.tile([128, QW], f32, name=f"vt{qr}", tag="vt")
                      for n in range(NB):
                          ps = prj.tile([128, 512], f32, name=f"pv{qr}{n}", tag="prj")
                          for k in range(8):
                              nc.tensor.matmul(ps[:, :], wv_sb[:, k, :], xts[k][:, n * 512:(n + 1) * 512],
                                               start=(k == 0), stop=(k == 7))
                          nc.scalar.copy(vt_sb[:, n * 512:(n + 1) * 512], ps[:, :])
                      for rt in range(QW // 128):
                          gr = c0 + rt * 128                    # global row
                          sk = (gr % S) // 128                  # key tile within batch
                          vb = v_sb[gr // S]
                          tp = tpp.tile([128, 128], f32, name=f"tp{qr}{rt}", tag="tp")
                          nc.tensor.transpose(tp[:, :], vt_sb[:, rt * 128:(rt + 1) * 128], id_sb[:, :])
                          dst = vb[:, sk * 130: sk * 130 + 130].rearrange("p (h c) -> p h c", c=65)
                          nc.vector.tensor_copy(dst[:, :, 0:64],
                                                tp[:, :].rearrange("p (h c) -> p h c", c=64))

              # ---- attention + output projection ----
              rctx = contextlib.ExitStack()
              sc_ps = rctx.enter_context(tc.tile_pool(name=f"sc_ps{_rep}", bufs=2, space="PSUM"))
              pv_ps = rctx.enter_context(tc.tile_pool(name=f"pv_ps{_rep}", bufs=2, space="PSUM"))
              ax_ps = rctx.enter_context(tc.tile_pool(name=f"ax_ps{_rep}", bufs=2, space="PSUM"))
              ysb_pool = rctx.enter_context(tc.tile_pool(name=f"ysb{_rep}", bufs=2))
              def emit_outproj(pb, pqh, py_sb, rts):
                  pcc = pb * 2 + pqh
                  for rt in rts:
                      gr = pb * S + pqh * 1024 + rt * 128
                      for nb2 in range(2):
                          yp = ax_ps.tile([128, 512], f32, name=f"yp{_rep}{pcc}{rt}{nb2}", tag="axp")
                          nc.tensor.matmul(yp[:, :], attn[pb][:, (gr % S):(gr % S) + 128],
                                           wo_sb[:, nb2 * 512:(nb2 + 1) * 512],
                                           start=True, stop=True)
                          nc.vector.tensor_copy(py_sb[:, rt, nb2 * 512:(nb2 + 1) * 512], yp[:, :])

              def emit_ydma(pb, pqh, py_sb, fine=False):
                  r0 = pb * S + pqh * 1024
                  n = 8 if fine else 4
                  w = 8 // n
                  for hh in range(n):
                      rr = r0 + hh * 128 * w
                      nc.sync.dma_start(
                          out=yo[rr:rr + 128 * w, :].rearrange("(t p) m -> p t m", p=128),
                          in_=py_sb[:, hh * w:(hh + 1) * w, :])

              pending = None   # previous chunk's outproj, interleaved into this one
              for b in range(B):
                  for qh in range(2):              # row-chunk of 1024 (4 q-blocks)
                      cc = b * 2 + qh
                      y_sb = ysb_pool.tile([128, 8, D], bf16, name=f"ysb{_rep}{cc}", tag="ysb")
                      group_idx = 0
                      for h in (1, 0):             # h=1 first: its attn DMA overlaps
                          for qp in range(2):      # h=0 compute
                              qbs = (qh * 4 + qp * 2, qh * 4 + qp * 2 + 1)
                              pvt = pv_ps.tile([65, 512], f32, name=f"pv{_rep}{cc}{h}{qp}", tag="pv")
                              pv = pvt[0:65, :]
                              for qi, qb in enumerate(qbs):
                                  nsk = 2 * (qb + 1)
                                  q_sl = slice(b * S + qb * 256, b * S + (qb + 1) * 256)
                                  # 2nd diagonal tile (sk=2qb+1): its first 128 q
                                  # cols are fully masked — compute only the valid
                                  # half (qb>=1; qb=0 keeps the plain path)
                                  for ch0 in range(0, nsk, 4):
                                      m = min(4, nsk - ch0)
                                      last_chunk = (ch0 + m == nsk)
                                      trim = qb >= 1 and last_chunk
                                      sc = sc_ps.tile([128, 1024], f32, name=f"sc{_rep}{cc}{h}{qp}{qi}{ch0}", tag="sc")
                                      for j in range(m):
                                          sk = ch0 + j
                                          k_sl = slice(b * S + sk * 128, b * S + (sk + 1) * 128)
                                          if trim and sk == 2 * qb + 1:
                                              qh_sl = slice(b * S + qb * 256 + 128,
                                                            b * S + (qb + 1) * 256)
                                              nc.tensor.matmul(sc[:, j * 256:j * 256 + 128],
                                                               k_rope[64 * h:64 * h + 64, k_sl],
                                                               q_rope[64 * h:64 * h + 64, qh_sl],
                                                               start=True, stop=True)
                                          else:
                                              o = slice(j * 256, (j + 1) * 256)
                                              nc.tensor.matmul(sc[:, o], k_rope[64 * h:64 * h + 64, k_sl],
                                                               q_rope[64 * h:64 * h + 64, q_sl],
                                                               start=True, stop=True)
                                      ncols = (m - 1) * 256 + 128 if trim else m * 256
                                      ex = exp_pool.tile([128, 1024], bf16, name=f"ex{_rep}{cc}{h}{qp}{qi}{ch0}", tag="ex")
                                      nc.scalar.activation(ex[:, 0:ncols], sc[:, 0:ncols],
                                                           mybir.ActivationFunctionType.Exp, scale=SCALE)
                                      for j in range(m):
                                          sk = ch0 + j
                                          o = slice(j * 256, (j + 1) * 256)
                                          if sk == 2 * qb:      # diagonal masking
                                              nc.vector.tensor_tensor(ex[:, o], ex[:, o], tri_sb[:, 0:256],
                                                                      mybir.AluOpType.mult)
                                          elif sk == 2 * qb + 1:
                                              if trim:
                                                  oh = slice(j * 256, j * 256 + 128)
                                                  nc.vector.tensor_tensor(ex[:, oh], ex[:, oh],
                                                                          tri_sb[:, 0:128],
                                                                          mybir.AluOpType.mult)
                                              else:
                                                  nc.vector.tensor_tensor(ex[:, o], ex[:, o], tri_sb[:, 256:512],
                                                                          mybir.AluOpType.mult)
                                      # PV: non-diag tiles, then the half-width 2nd
                                      # diagonal, then the full-width 1st diagonal
                                      # carrying the stop flag (accumulation is
                                      # commutative; stop must land on a matmul
                                      # covering every accumulated column)
                                      if trim:
                                          js = ([j for j in range(m) if ch0 + j < 2 * qb]
                                                + [m - 1, m - 2])
                                      else:
                                          js = ([j for j in range(m) if ch0 + j < 2 * qb]
                                                + [j for j in range(m) if ch0 + j >= 2 * qb])
                                      for j in js:
                                          sk = ch0 + j
                                          if trim and sk == 2 * qb + 1:
                                              nc.tensor.matmul(pv[:, qi * 256 + 128:(qi + 1) * 256],
                                                               v_sb[b][:, sk * 130 + 65 * h: sk * 130 + 65 * h + 65],
                                                               ex[:, j * 256:j * 256 + 128],
                                                               start=False, stop=False)
                                          else:
                                              o = slice(j * 256, (j + 1) * 256)
                                              stop = (sk == 2 * qb) if trim else (sk == nsk - 1)
                                              nc.tensor.matmul(pv[:, qi * 256:(qi + 1) * 256],
                                                               v_sb[b][:, sk * 130 + 65 * h: sk * 130 + 65 * h + 65],
                                                               ex[:, o],
                                                               start=(sk == 0), stop=stop)
                              # normalize: attn rows = pv_vals * (1/rowsum broadcast)
                              rec = aux.tile([1, 512], f32r, name=f"rec{_rep}{cc}{h}{qp}", tag="rec")
                              with nc.allow_low_precision(reason="softmax reciprocal"):
                                  nc.vector.reciprocal(rec[0:1, :], pv[64:65, :])
                              bc = aux.tile([64, 512], f32r, name=f"bc{_rep}{cc}{h}{qp}", tag="bc")
                              nc.gpsimd.partition_broadcast(bc[0:64, :], rec[0:1, :], channels=64)
                              a_sl = slice((qh * 2 + qp) * 512, (qh * 2 + qp + 1) * 512)
                              if h == 0:
                                  nc.vector.tensor_tensor(attn[b][0:64, a_sl], pv[0:64, :], bc[0:64, :],
                                                          mybir.AluOpType.mult)
                              else:
                                  hs_q = aux.tile([64, 512], f32r, name=f"hs{_rep}{cc}{qp}", tag="hs")
                                  nc.vector.tensor_tensor(hs_q[0:64, :], pv[0:64, :], bc[0:64, :],
                                                          mybir.AluOpType.mult)
                                  nc.sync.dma_start(out=attn[b][64:128, a_sl], in_=hs_q[0:64, :])
                              # interleave previous chunk's output projection into
                              # this chunk's Act-bound stalls (2 of its 8 row-tiles
                              # per group)
                              if pending is not None:
                                  pb, pqh, py_sb = pending
                                  emit_outproj(pb, pqh, py_sb,
                                               range(group_idx * 2, group_idx * 2 + 2))
                              group_idx += 1
                      if pending is not None:
                          emit_ydma(*pending)
                      pending = (b, qh, y_sb)
              # flush the last chunk's output projection
              emit_outproj(*pending, rts=range(8))
              emit_ydma(*pending, fine=True)
              rctx.close()
    nc.finalize()
    return nc


# ---------------------------------------------------------------------------
# host-side weight prelayout (cheap numpy; ~8 MB bf16 total)
# ---------------------------------------------------------------------------

def _weight_blob(Wq, Wk, Wv, Wo):
    """Per-core [128, 4096] f32: [wq | wk | wv | wo] columns; stacked to
    [1024, 4096] for P('core') sharding."""
    blob = np.empty((NC * 128, 4 * D), dtype=np.float32)
    for c in range(NC):
        h0 = HPC * c
        rows = []
        for j in range(HPC):
            rows += [(h0 + j) * DH + 2 * i for i in range(DH // 2)]      # evens
            rows += [(h0 + j) * DH + 2 * i + 1 for i in range(DH // 2)]  # odds

        def _prelayout(w):                                               # [D,128] -> [128, 8*128]
            return w.reshape(8, 128, 128).transpose(1, 0, 2).reshape(128, 8 * 128)

        r0 = c * 128
        blob[r0:r0 + 128, 0 * D:1 * D] = _prelayout(Wq[rows, :].T)
        blob[r0:r0 + 128, 1 * D:2 * D] = _prelayout(Wk[rows, :].T)
        vrows = list(range(h0 * DH, (h0 + HPC) * DH))
        blob[r0:r0 + 128, 2 * D:3 * D] = _prelayout(Wv[vrows, :].T)
        blob[r0:r0 + 128, 3 * D:4 * D] = Wo[:, vrows].T                  # [128, D]
    return blob


# ---------------------------------------------------------------------------
# cached jits
# ---------------------------------------------------------------------------

# fused uint8 upload layout (block-float9: int8 mantissa + shared uint8
# exponent per 8 consecutive elements -> 1.125 B/el, accuracy ~ bf16).
# x and weight mantissa planes are contiguous so the device decode is a
# single [1024, 1024] pass (two decode subgraphs in one XLA module ICE
# the neuron compiler).
_XQ = (R // NC) * D          # 524288 x mantissa bytes per core
_WQ = 128 * 4 * D            # 524288 weight-blob mantissa bytes per core
_MQ = _XQ + _WQ              # combined mantissa plane
_ME = _MQ // 8               # 131072 combined exponent bytes (x first)
_PN = 3 * S                  # 6144 position-limb bytes
_L8 = _MQ + _ME + _PN


def _enc_bf9(x):
    """x f32 [..., n*8] -> (q int8 flat bytes view, e uint8 flat)."""
    xr = x.reshape(-1, 8)
    bm = np.abs(xr).max(1)
    e = np.frexp(bm)[1]                                   # bm = f*2^e, f in [.5,1)
    factor = np.ldexp(np.float32(1.0), 7 - e).astype(np.float32)
    q = np.clip(np.rint(xr * factor[:, None]), -127, 127).astype(np.int8)
    return q.view(np.uint8).ravel(), (e + 127).astype(np.uint8).ravel()


def _dec_bf9_host(qbytes, e, shape):
    q = (qbytes.astype(np.float32) - 128.0).reshape(-1, 8)   # excess-128 uint8
    f = np.ldexp(np.float32(1.0), e.astype(np.int32) - 133).astype(np.float32)
    return (q * f.reshape(-1, 1)).reshape(shape)


def _setup():
    nc = _build()
    devs = jax.devices()[:NC]
    mesh = Mesh(np.asarray(devs), ("core",))
    sh_core = NamedSharding(mesh, P("core"))

    # --- prep jit: decode bf9, all-gather x, transpose, RoPE tables ---
    inv_np = (THETA ** (-np.arange(0, DH, 2, dtype=np.float64) / DH)).astype(np.float32)

    def _dec(qu8, e, r, c):
        q = jax.lax.bitcast_convert_type(qu8.reshape(r, c), jnp.int8).astype(jnp.float32)
        f = jnp.exp2(e.reshape(r, c // 8).astype(jnp.float32) - 134.0)
        return q * jnp.repeat(f, 8, axis=1)

    # stage 1: pure bf9 decode (fused uint8 -> xl bf16, bl f32, pos f32)
    def _decode(fl):
        # fl [1, _L8] uint8: x|w mantissas, x|w exponents, position limbs
        m = fl[0, :_MQ]
        e = fl[0, _MQ:_MQ + _ME]
        limbs = fl[0, _MQ + _ME:].reshape(3, S).astype(jnp.float32)   # exact ints < 256
        pos = (limbs[0] * 65536.0 + limbs[1] * 256.0 + limbs[2]).reshape(1, S)
        full = _dec(m, e, (R // NC) + 128 * 4, D)                     # [1024, 1024] f32
        xl = full[:R // NC].astype(jnp.bfloat16)                      # [512, 1024]
        bl = full[R // NC:].reshape(128, 4 * D)                       # [128, 4096] f32
        return xl, bl, pos

    decode = jax.jit(shard_map(
        _decode, mesh=mesh, in_specs=(P("core"),),
        out_specs=(P("core"),) * 3, check_rep=False))

    # stage 2: all-gather x, transpose, RoPE tables, weight split
    def _prep(xl, bl, pl):
        pos = pl[0]                                                   # [2048] f32
        xfull = jax.lax.all_gather(xl, "core", axis=0, tiled=True)    # [4096, 1024]
        xt = xfull.T                                                  # [1024, 4096] bf16
        ang = inv_np[:, None] * pos[None, :]                          # [32, 2048]
        cos32 = jnp.cos(ang)
        sin32 = jnp.sin(ang)
        cost = jnp.concatenate([cos32] * 4, axis=0)                   # [128, S] f32
        sint = jnp.concatenate([-sin32, sin32, -sin32, sin32], axis=0).astype(jnp.bfloat16)
        wq = bl[:, 0 * D:1 * D].astype(jnp.bfloat16)
        wk = bl[:, 1 * D:2 * D].astype(jnp.bfloat16)
        wv = bl[:, 2 * D:3 * D].astype(jnp.bfloat16)
        wo = bl[:, 3 * D:4 * D]                                       # f32
        return xt, wq, wk, wv, wo, cost, sint

    prep = jax.jit(shard_map(
        _prep, mesh=mesh,
        in_specs=(P("core"),) * 3,
        out_specs=(P("core"),) * 7, check_rep=False))

    # --- bass jit (adapted from bass2jax.run_bass_via_pjrt, cached + no
    #     donated zero outputs) ---
    b2j.install_neuronx_cc_hook()
    partition_name = nc.partition_id_tensor.name if nc.partition_id_tensor else None
    in_names, out_names, out_avals = [], [], []
    for alloc in nc.m.functions[0].allocations:
        if not isinstance(alloc, mybir.MemoryLocationSet):
            continue
        name = alloc.memorylocations[0].name
        if alloc.kind == "ExternalInput":
            if name != partition_name:
                in_names.append(name)
        elif alloc.kind == "ExternalOutput":
            out_names.append(name)
            out_avals.append(jax.core.ShapedArray(
                tuple(alloc.tensor_shape), mybir.dt.np(alloc.dtype)))

    def _body(*args):
        operands = list(args)
        if partition_name is not None:
            operands.append(b2j.partition_id_tensor())
        outs = b2j._bass_exec_p.bind(
            *operands,
            out_avals=tuple(out_avals),
            in_names=tuple(in_names) + ((partition_name,) if partition_name else ()),
            out_names=tuple(out_names),
            lowering_input_output_aliases=(),
            sim_require_finite=True,
            sim_require_nnan=True,
            nc=nc,
        )
        return tuple(outs)

    bass_call = jax.jit(shard_map(
        _body, mesh=mesh,
        in_specs=(P("core"),) * len(in_names),
        out_specs=(P("core"),) * len(out_names), check_rep=False))

    # --- reduce jit: on-device 8-way partial sum + bf9 encode (4.5 MB).
    # Mantissas are stored excess-128 (q+128 as uint8): the device's
    # f32->int8 conversion saturates negatives to 0. ---
    def _reduce(yl):
        y = jax.lax.psum(yl.astype(jnp.float32), "core")              # [4096, 1024]
        r, c = y.shape
        bm = jnp.maximum(jnp.max(jnp.abs(y.reshape(r, c // 8, 8)), axis=2), 1e-8)
        b = (jax.lax.bitcast_convert_type(bm, jnp.int32) >> 23) & 0xFF
        factor = jnp.exp2(133.0 - b.astype(jnp.float32))
        q = jnp.clip(jnp.round(y * jnp.repeat(factor, 8, axis=1)) + 128.0,
                     1, 255).astype(jnp.uint8)
        return jnp.concatenate([q, b.astype(jnp.uint8)], axis=1)      # [4096, 1152]

    reduce = jax.jit(shard_map(
        _reduce, mesh=mesh, in_specs=(P("core"),), out_specs=P(),
        check_rep=False))

    # --- device-resident constants (one-time 1.7 MB upload) ---
    ones_c = jax.device_put(np.ones((NC * 128, DH), np.float32), sh_core)
    ident_c = jax.device_put(
        np.tile(np.eye(128, dtype=np.float32), (NC, 1)), sh_core)
    kk = np.arange(128)[:, None]
    qq = np.arange(256)[None, :]
    tri = np.concatenate([(qq >= kk).astype(npbf),
                          (qq - 128 >= kk).astype(npbf)], 1)          # [128, 512]
    trim_c = jax.device_put(np.tile(tri, (NC, 1)), sh_core)

    return dict(mesh=mesh, sh_core=sh_core, decode=decode, prep=prep,
                bass_call=bass_call, reduce=reduce, ones_c=ones_c,
                ident_c=ident_c, trim_c=trim_c, in_names=in_names)


def kernel(in_features, token_positions, Wq, Wk, Wv, Wo):
    if "ctx" not in _CACHE:
        _CACHE["ctx"] = _setup()
    C = _CACHE["ctx"]
    sh_core = C["sh_core"]

    # one fused upload [8, _L8] uint8 (~9 MB): x|w bf9 mantissas, exponents,
    # pos limbs
    fused = np.empty((NC, _L8), dtype=np.uint8)
    xq, xe = _enc_bf9(np.asarray(in_features, dtype=np.float32).reshape(R, D))
    fused[:, :_XQ] = xq.reshape(NC, _XQ)
    blob = _weight_blob(np.asarray(Wq, np.float32), np.asarray(Wk, np.float32),
                        np.asarray(Wv, np.float32), np.asarray(Wo, np.float32))
    wqb, web = _enc_bf9(blob)
    fused[:, _XQ:_MQ] = wqb.reshape(NC, _WQ)
    fused[:, _MQ:_MQ + _XQ // 8] = xe.reshape(NC, _XQ // 8)
    fused[:, _MQ + _XQ // 8:_MQ + _ME] = web.reshape(NC, _WQ // 8)
    p = np.asarray(token_positions, np.int64)
    limbs = np.stack([(p >> 16) & 255, (p >> 8) & 255, p & 255]).astype(np.uint8)
    fused[:, _MQ + _ME:] = limbs.reshape(1, _PN)

    fd = jax.device_put(fused, sh_core)       # single RPC

    xl, bl, pl = C["decode"](fd)
    xt, wq, wk, wv, wo, cost, sint = C["prep"](xl, bl, pl)
    (yo,) = C["bass_call"](xt, wq, wk, wv, wo, cost, sint,
                           C["ones_c"], C["ident_c"], C["trim_c"])
    packed = np.asarray(C["reduce"](yo))      # [4096, 1152] uint8, 4.5 MB
    y = _dec_bf9_host(np.ascontiguousarray(packed[:, :D]), packed[:, D:], (R, D))
    return y.reshape(B, S, D)


# revision 19
# speedup vs baseline: 1.1070x; 1.1070x over previous
"""Causal multi-head self-attention (B=2, S=2048, D=1024, H=16) on 8 trn2 cores.

Device kernel (unchanged from v1): tensor-parallel over heads — core c owns
heads (2c, 2c+1), both batches, full sequence. Per core: QKV projections for
its 2 heads, RoPE, causal attention, output-projection partial product into
yo [4096, 1024] bf16.

Dispatch layer (v2): the wall-clock metric is dominated by the ~50 MB/s
axon tunnel, so per-call host<->device traffic is cut from ~207 MB to
~16 MB up + 8 MB down:
  - x ships ONCE row-sharded (bf16, 1 MB/core); an XLA prep jit all-gathers
    it on device over NeuronLink, transposes to xt [D, R], and builds the
    RoPE cos/sin tables on device from token_positions (so the 12 MB of
    per-core tables never cross the tunnel).
  - per-core weight slices ship as one bf16 blob (8 MB total; wo is cast
    back to f32 on device).
  - no donated zero output buffers (v1 shipped 64 MB of zeros per call;
    yo is fully written by the kernel so uninit PJRT allocation is fine).
  - the 8 partial yo's are psum-reduced ON DEVICE (f32 accumulate) and a
    single bf16 [4096, 1024] replica is fetched (8 MB vs 64 MB).
  - all three jits (prep / bass_exec / reduce) are built once and cached;
    v1 re-traced + re-lowered (incl. zstd of the BIR) every call.
"""
import math
import numpy as np

import jax
import jax.numpy as jnp
from jax.sharding import Mesh, PartitionSpec as P, NamedSharding

from jax.experimental.shard_map import shard_map

import ml_dtypes

import concourse.bass as bass
from concourse import bacc
import concourse.mybir as mybir
from concourse.tile import TileContext
from concourse import bass2jax as b2j

THETA = 10000.0
B, S, D, H = 2, 2048, 1024, 16
DH = D // H          # 64
NC = 8               # cores
HPC = H // NC        # heads per core = 2
R = B * S            # 4096 flat rows
SCALE = 1.0 / math.sqrt(DH)

f32 = mybir.dt.float32
f32r = mybir.dt.float32r
bf16 = mybir.dt.bfloat16
npbf = ml_dtypes.bfloat16

_CACHE = {}


def _build(_DBG_REPS=1):
    nc = bacc.Bacc(num_devices=NC)

    xt = nc.declare_dram_parameter("xt", [D, R], bf16, isOutput=False)
    wq = nc.declare_dram_parameter("wq", [128, 8 * 2 * DH], bf16, isOutput=False)
    wk = nc.declare_dram_parameter("wk", [128, 8 * 2 * DH], bf16, isOutput=False)
    wv = nc.declare_dram_parameter("wv", [128, 8 * 2 * DH], bf16, isOutput=False)
    wo = nc.declare_dram_parameter("wo", [2 * DH, D], f32r, isOutput=False)
    cost = nc.declare_dram_parameter("cost", [128, S], f32, isOutput=False)
    sint = nc.declare_dram_parameter("sint", [128, S], bf16, isOutput=False)
    ones = nc.declare_dram_parameter("ones", [128, DH], f32r, isOutput=False)
    ident = nc.declare_dram_parameter("ident", [128, 128], f32, isOutput=False)
    trim = nc.declare_dram_parameter("trim", [128, 512], bf16, isOutput=False)
    yo = nc.declare_dram_parameter("yo", [R, D], bf16, isOutput=True)

    NQ = 4            # xT column quarters in phase A
    QW = R // NQ      # 1024 rows per quarter
    NB = QW // 512    # 2 proj psum blocks per quarter
    KT = S // 128     # 16 key tiles per batch

    with TileContext(nc) as tc:
        import contextlib
        ctx = contextlib.ExitStack()
        with ctx:
            # ---- persistent pools (whole kernel) ----
            pers = ctx.enter_context(tc.tile_pool(name="pers", bufs=1))
            exp_pool = ctx.enter_context(tc.tile_pool(name="expp", bufs=8))
            aux = ctx.enter_context(tc.tile_pool(name="aux", bufs=2))

            q_rope = pers.tile([128, R], bf16, name="q_rope")
            k_rope = pers.tile([128, R], bf16, name="k_rope")
            # V per batch, natural rows layout; per key tile 130 cols:
            #   per head: [v(64) | one]
            v_sb = [pers.tile([128, KT * 130], bf16, name=f"v_sb{b}") for b in range(B)]
            attn = [pers.tile([128, S], f32r, name=f"attn{b}") for b in range(B)]
            wo_sb = pers.tile([128, D], f32r, name="wo_sb")
            ones_sb = pers.tile([128, DH], f32r, name="ones_sb")
            id_sb = pers.tile([128, 128], f32, name="id_sb")
            tri_sb = pers.tile([128, 512], bf16, name="tri_sb")
            for b in range(B):
                nc.vector.memset(
                    v_sb[b][:, :].rearrange("p (t c) -> p t c", c=65)[:, :, 64:65], 1.0)

            # ---- phase A: projections + RoPE + V assembly ----
            for _rep in range(_DBG_REPS):
              with tc.tile_pool(name="phA", bufs=1) as pha, \
                   tc.tile_pool(name="xtp", bufs=2) as xtp, \
                   tc.tile_pool(name="ropetmp", bufs=2) as rtp, \
                   tc.tile_pool(name="prj_ps", bufs=6, space="PSUM") as prj, \
                   tc.tile_pool(name="tp_ps", bufs=2, space="PSUM") as tpp:

                  wq_sb = pha.tile([128, 8, 2 * DH], bf16, name="wq_sb")
                  wk_sb = pha.tile([128, 8, 2 * DH], bf16, name="wk_sb")
                  wv_sb = pha.tile([128, 8, 2 * DH], bf16, name="wv_sb")
                  cos_sb = pha.tile([128, S], f32, name="cos_sb")
                  sin_sb = pha.tile([128, S], bf16, name="sin_sb")
                  nc.sync.dma_start(out=wq_sb[:, :, :], in_=wq.rearrange("p (t m) -> p t m", t=8))
                  xtb0 = [xtp.tile([128, 4, QW], bf16, name=f"xt0{hh}", tag=f"xt{hh}")
                          for hh in range(2)]
                  for hh in range(2):
                      for sub in range(2):
                          nc.sync.dma_start(
                              out=xtb0[hh][:, sub * 2:(sub + 1) * 2, :],
                              in_=xt[hh * 512 + sub * 256:hh * 512 + (sub + 1) * 256,
                                     0:QW].rearrange("(t p) m -> p t m", p=128))
                  nc.sync.dma_start(out=wk_sb[:, :, :], in_=wk.rearrange("p (t m) -> p t m", t=8))
                  nc.sync.dma_start(out=wv_sb[:, :, :], in_=wv.rearrange("p (t m) -> p t m", t=8))
                  nc.sync.dma_start(out=cos_sb[:, :], in_=cost[:, :])
                  nc.sync.dma_start(out=sin_sb[:, :], in_=sint[:, :])
                  if _rep == 0:
                      nc.sync.dma_start(out=id_sb[:, :], in_=ident[:, :])
                      nc.sync.dma_start(out=tri_sb[:, :], in_=trim[:, :])
                      nc.sync.dma_start(out=ones_sb[:, :], in_=ones[:, :])
                      nc.sync.dma_start(out=wo_sb[:, :], in_=wo[:, :])

                  for qr in range(NQ):
                      c0 = qr * QW           # global row offset of this quarter
                      s0 = c0 % S            # seq offset of this quarter
                      if qr == 0:
                          xtb = xtb0
                      else:
                          xtb = xtb_next
                      if qr + 1 < NQ:
                          cn = (qr + 1) * QW
                          xtb_next = [xtp.tile([128, 4, QW], bf16, name=f"xt{qr + 1}{hh}", tag=f"xt{hh}")
                                      for hh in range(2)]
                          for hh in range(2):
                              nc.sync.dma_start(
                                  out=xtb_next[hh][:, :, :],
                                  in_=xt[hh * 512:(hh + 1) * 512, cn:cn + QW].rearrange(
                                      "(t p) m -> p t m", p=128))
                      xts = [xtb[k // 4][:, k % 4, :] for k in range(8)]

                      for tname, wsb, rope in (("q", wq_sb, q_rope), ("k", wk_sb, k_rope)):
                          pss = []
                          for n in range(NB):
                              ps = prj.tile([128, 512], f32, name=f"p{tname}{qr}{n}", tag="prj")
                              for k in range(8):
                                  nc.tensor.matmul(ps[:, :], wsb[:, k, :], xts[k][:, n * 512:(n + 1) * 512],
                                                   start=(k == 0), stop=(k == 7))
                              pss.append(ps)
                          # RoPE: per head rows [even(32) | odd(32)]; sin table
                          # carries the sign, so out = ps*cos + swap32(ps)*sin.
                          raw = rtp.tile([128, QW], bf16, name=f"raw{tname}", tag="raw")
                          pcs = rtp.tile([128, QW], bf16, name=f"pcs{tname}", tag="pcs")
                          for n in range(NB):
                              cs = slice(n * 512, (n + 1) * 512)
                              gs = slice(s0 + n * 512, s0 + (n + 1) * 512)
                              nc.vector.tensor_copy(raw[:, cs], pss[n][:, :])
                              nc.vector.tensor_tensor(pcs[:, cs], pss[n][:, :], cos_sb[:, gs], mybir.AluOpType.mult)
                          swp = rtp.tile([128, QW], bf16, name=f"swp{tname}", tag="swp")
                          for r0 in range(0, 128, 64):
                              nc.sync.dma_start(out=swp[r0:r0 + 32, :], in_=raw[r0 + 32:r0 + 64, :])
                              nc.sync.dma_start(out=swp[r0 + 32:r0 + 64, :], in_=raw[r0:r0 + 32, :])
                          gq = slice(s0, s0 + QW)
                          tsn = rtp.tile([128, QW], bf16, name=f"tsn{tname}", tag="tsn")
                          nc.vector.tensor_tensor(tsn[:, :], swp[:, :], sin_sb[:, gq], mybir.AluOpType.mult)
                          nc.vector.tensor_tensor(rope[:, c0:c0 + QW], pcs[:, :], tsn[:, :], mybir.AluOpType.add)

                      # V: transposed projection (N=512) then PE-transpose to natural
                      vt_sb = rtp.tile([128, QW], f32, name=f"vt{qr}", tag="vt")
                      for n in range(NB):
                          ps = prj.tile([128, 512], f32, name=f"pv{qr}{n}", tag="prj")
                          for k in range(8):
                              nc.tensor.matmul(ps[:, :], wv_sb[:, k, :], xts[k][:, n * 512:(n + 1) * 512],
                                               start=(k == 0), stop=(k == 7))
                          nc.scalar.copy(vt_sb[:, n * 512:(n + 1) * 512], ps[:, :])
                      for rt in range(QW // 128):
                          gr = c0 + rt * 128                    # global row
                          sk = (gr % S) // 128                  # key tile within batch
                          vb = v_sb[gr // S]
                          tp = tpp.tile([128, 128], f32, name=f"tp{qr}{rt}", tag="tp")
                          nc.tensor.transpose(tp[:, :], vt_sb[:, rt * 128:(rt + 1) * 128], id_sb[:, :])
                          dst = vb[:, sk * 130: sk * 130 + 130].rearrange("p (h c) -> p h c", c=65)
                          nc.vector.tensor_copy(dst[:, :, 0:64],
                                                tp[:, :].rearrange("p (h c) -> p h c", c=64))

              # ---- attention + output projection ----
              rctx = contextlib.ExitStack()
              sc_ps = rctx.enter_context(tc.tile_pool(name=f"sc_ps{_rep}", bufs=2, space="PSUM"))
              pv_ps = rctx.enter_context(tc.tile_pool(name=f"pv_ps{_rep}", bufs=2, space="PSUM"))
              ax_ps = rctx.enter_context(tc.tile_pool(name=f"ax_ps{_rep}", bufs=2, space="PSUM"))
              ysb_pool = rctx.enter_context(tc.tile_pool(name=f"ysb{_rep}", bufs=2))
              def emit_outproj(pb, pqh, py_sb, rts):
                  pcc = pb * 2 + pqh
                  for rt in rts:
                      gr = pb * S + pqh * 1024 + rt * 128
                      for nb2 in range(2):
                          yp = ax_ps.tile([128, 512], f32, name=f"yp{_rep}{pcc}{rt}{nb2}", tag="axp")
                          nc.tensor.matmul(yp[:, :], attn[pb][:, (gr % S):(gr % S) + 128],
                                           wo_sb[:, nb2 * 512:(nb2 + 1) * 512],
                                           start=True, stop=True)
                          nc.vector.tensor_copy(py_sb[:, rt, nb2 * 512:(nb2 + 1) * 512], yp[:, :])

              def emit_ydma(pb, pqh, py_sb, fine=False):
                  r0 = pb * S + pqh * 1024
                  n = 8 if fine else 4
                  w = 8 // n
                  for hh in range(n):
                      rr = r0 + hh * 128 * w
                      nc.sync.dma_start(
                          out=yo[rr:rr + 128 * w, :].rearrange("(t p) m -> p t m", p=128),
                          in_=py_sb[:, hh * w:(hh + 1) * w, :])

              pending = None   # previous chunk's outproj, interleaved into this one
              for b in range(B):
                  for qh in range(2):              # row-chunk of 1024 (4 q-blocks)
                      cc = b * 2 + qh
                      y_sb = ysb_pool.tile([128, 8, D], bf16, name=f"ysb{_rep}{cc}", tag="ysb")
                      group_idx = 0
                      for h in (1, 0):             # h=1 first: its attn DMA overlaps
                          for qp in range(2):      # h=0 compute
                              qbs = (qh * 4 + qp * 2, qh * 4 + qp * 2 + 1)
                              pvt = pv_ps.tile([65, 512], f32, name=f"pv{_rep}{cc}{h}{qp}", tag="pv")
                              pv = pvt[0:65, :]
                              for qi, qb in enumerate(qbs):
                                  nsk = 2 * (qb + 1)
                                  q_sl = slice(b * S + qb * 256, b * S + (qb + 1) * 256)
                                  # 2nd diagonal tile (sk=2qb+1): its first 128 q
                                  # cols are fully masked — compute only the valid
                                  # half (qb>=1; qb=0 keeps the plain path)
                                  for ch0 in range(0, nsk, 4):
                                      m = min(4, nsk - ch0)
                                      last_chunk = (ch0 + m == nsk)
                                      trim = qb >= 1 and last_chunk
                                      sc = sc_ps.tile([128, 1024], f32, name=f"sc{_rep}{cc}{h}{qp}{qi}{ch0}", tag="sc")
                                      for j in range(m):
                                          sk = ch0 + j
                                          k_sl = slice(b * S + sk * 128, b * S + (sk + 1) * 128)
                                          if trim and sk == 2 * qb + 1:
                                              qh_sl = slice(b * S + qb * 256 + 128,
                                                            b * S + (qb + 1) * 256)
                                              nc.tensor.matmul(sc[:, j * 256:j * 256 + 128],
                                                               k_rope[64 * h:64 * h + 64, k_sl],
                                                               q_rope[64 * h:64 * h + 64, qh_sl],
                                                               start=True, stop=True)
                                          else:
                                              o = slice(j * 256, (j + 1) * 256)
                                              nc.tensor.matmul(sc[:, o], k_rope[64 * h:64 * h + 64, k_sl],
                                                               q_rope[64 * h:64 * h + 64, q_sl],
                                                               start=True, stop=True)
                                      ncols = (m - 1) * 256 + 128 if trim else m * 256
                                      ex = exp_pool.tile([128, 1024], bf16, name=f"ex{_rep}{cc}{h}{qp}{qi}{ch0}", tag="ex")
                                      nc.scalar.activation(ex[:, 0:ncols], sc[:, 0:ncols],
                                                           mybir.ActivationFunctionType.Exp, scale=SCALE)
                                      for j in range(m):
                                          sk = ch0 + j
                                          o = slice(j * 256, (j + 1) * 256)
                                          if sk == 2 * qb:      # diagonal masking
                                              nc.vector.tensor_tensor(ex[:, o], ex[:, o], tri_sb[:, 0:256],
                                                                      mybir.AluOpType.mult)
                                          elif sk == 2 * qb + 1:
                                              if trim:
                                                  oh = slice(j * 256, j * 256 + 128)
                                                  nc.vector.tensor_tensor(ex[:, oh], ex[:, oh],
                                                                          tri_sb[:, 0:128],
                                                                          mybir.AluOpType.mult)
                                              else:
                                                  nc.vector.tensor_tensor(ex[:, o], ex[:, o], tri_sb[:, 256:512],
                                                                          mybir.AluOpType.mult)
                                      # PV: non-diag tiles, then the half-width 2nd
                                      # diagonal, then the full-width 1st diagonal
                                      # carrying the stop flag (accumulation is
                                      # commutative; stop must land on a matmul
                                      # covering every accumulated column)
                                      if trim:
                                          js = ([j for j in range(m) if ch0 + j < 2 * qb]
                                                + [m - 1, m - 2])
                                      else:
                                          js = ([j for j in range(m) if ch0 + j < 2 * qb]
                                                + [j for j in range(m) if ch0 + j >= 2 * qb])
                                      for j in js:
                                          sk = ch0 + j
                                          if trim and sk == 2 * qb + 1:
                                              nc.tensor.matmul(pv[:, qi * 256 + 128:(qi + 1) * 256],
                                                               v_sb[b][:, sk * 130 + 65 * h: sk * 130 + 65 * h + 65],
                                                               ex[:, j * 256:j * 256 + 128],
                                                               start=False, stop=False)
                                          else:
                                              o = slice(j * 256, (j + 1) * 256)
                                              stop = (sk == 2 * qb) if trim else (sk == nsk - 1)
                                              nc.tensor.matmul(pv[:, qi * 256:(qi + 1) * 256],
                                                               v_sb[b][:, sk * 130 + 65 * h: sk * 130 + 65 * h + 65],
                                                               ex[:, o],
                                                               start=(sk == 0), stop=stop)
                              # normalize: attn rows = pv_vals * (1/rowsum broadcast)
                              rec = aux.tile([1, 512], f32r, name=f"rec{_rep}{cc}{h}{qp}", tag="rec")
                              with nc.allow_low_precision(reason="softmax reciprocal"):
                                  nc.vector.reciprocal(rec[0:1, :], pv[64:65, :])
                              bc = aux.tile([64, 512], f32r, name=f"bc{_rep}{cc}{h}{qp}", tag="bc")
                              nc.gpsimd.partition_broadcast(bc[0:64, :], rec[0:1, :], channels=64)
                              a_sl = slice((qh * 2 + qp) * 512, (qh * 2 + qp + 1) * 512)
                              if h == 0:
                                  nc.vector.tensor_tensor(attn[b][0:64, a_sl], pv[0:64, :], bc[0:64, :],
                                                          mybir.AluOpType.mult)
                              else:
                                  hs_q = aux.tile([64, 512], f32r, name=f"hs{_rep}{cc}{qp}", tag="hs")
                                  nc.vector.tensor_tensor(hs_q[0:64, :], pv[0:64, :], bc[0:64, :],
                                                          mybir.AluOpType.mult)
                                  nc.sync.dma_start(out=attn[b][64:128, a_sl], in_=hs_q[0:64, :])
                              # interleave previous chunk's output projection into
                              # this chunk's Act-bound stalls (2 of its 8 row-tiles
                              # per group)
                              if pending is not None:
                                  pb, pqh, py_sb = pending
                                  emit_outproj(pb, pqh, py_sb,
                                               range(group_idx * 2, group_idx * 2 + 2))
                              group_idx += 1
                      if pending is not None:
                          emit_ydma(*pending)
                      pending = (b, qh, y_sb)
              # flush the last chunk's output projection
              emit_outproj(*pending, rts=range(8))
              emit_ydma(*pending, fine=True)
              rctx.close()
    nc.finalize()
    return nc


# ---------------------------------------------------------------------------
# host-side weight prelayout (cheap numpy; ~8 MB bf16 total)
# ---------------------------------------------------------------------------

def _weight_blob(Wq, Wk, Wv, Wo):
    """Per-core [128, 4096] f32: [wq | wk | wv | wo] columns; stacked to
    [1024, 4096] for P('core') sharding."""
    blob = np.empty((NC * 128, 4 * D), dtype=np.float32)
    for c in range(NC):
        h0 = HPC * c
        rows = []
        for j in range(HPC):
            rows += [(h0 + j) * DH + 2 * i for i in range(DH // 2)]      # evens
            rows += [(h0 + j) * DH + 2 * i + 1 for i in range(DH // 2)]  # odds

        def _prelayout(w):                                               # [D,128] -> [128, 8*128]
            return w.reshape(8, 128, 128).transpose(1, 0, 2).reshape(128, 8 * 128)

        r0 = c * 128
        blob[r0:r0 + 128, 0 * D:1 * D] = _prelayout(Wq[rows, :].T)
        blob[r0:r0 + 128, 1 * D:2 * D] = _prelayout(Wk[rows, :].T)
        vrows = list(range(h0 * DH, (h0 + HPC) * DH))
        blob[r0:r0 + 128, 2 * D:3 * D] = _prelayout(Wv[vrows, :].T)
        blob[r0:r0 + 128, 3 * D:4 * D] = Wo[:, vrows].T                  # [128, D]
    return blob


# ---------------------------------------------------------------------------
# cached jits
# ---------------------------------------------------------------------------

# fused uint8 upload layout (block-float9: int8 mantissa + shared uint8
# exponent per 8 consecutive elements -> 1.125 B/el, accuracy ~ bf16).
# x and weight mantissa planes are contiguous so the device decode is a
# single [1024, 1024] pass (two decode subgraphs in one XLA module ICE
# the neuron compiler).
_XQ = (R // NC) * D          # 524288 x mantissa bytes per core
_WQ = 128 * 4 * D            # 524288 weight-blob mantissa bytes per core
_MQ = _XQ + _WQ              # combined mantissa plane
_ME = _MQ // 8               # 131072 combined exponent bytes (x first)
_PN = 3 * S                  # 6144 position-limb bytes
_L8 = _MQ + _ME + _PN


def _enc_bf9_chunk(x, qout, eout):
    """x f32 [n*8] contiguous -> int8 mantissas into qout, uint8 exps into
    eout. Exponent via f32 bit tricks (no frexp), factor = 2^(133-eb)."""
    xr = x.reshape(-1, 8)
    bm = np.abs(xr).max(1)
    eb = (bm.view(np.int32) >> 23) & 0xFF                 # biased exp of blockmax
    np.maximum(eb, 100, out=eb)                           # avoid 2^k overflow at bm=0
    factor = ((260 - eb) << 23).view(np.float32)          # 2^(133-eb)
    q = np.rint(xr * factor[:, None])
    np.clip(q, -127, 127, out=q)
    qout[:] = q.astype(np.int8).view(np.uint8).ravel()
    eout[:] = (eb + 1).astype(np.uint8)                   # = frexp_exp + 127


_POOL = None


def _pool():
    global _POOL
    if _POOL is None:
        from concurrent.futures import ThreadPoolExecutor
        _POOL = ThreadPoolExecutor(max_workers=8)
    return _POOL


def _enc_bf9_par(arrs_dsts, nchunk=4):
    """[(x_flat, q_dst, e_dst), ...] encoded in parallel chunks."""
    jobs = []
    for x, qd, ed in arrs_dsts:
        n = x.size
        step = n // nchunk
        assert step % 8 == 0
        for i in range(nchunk):
            s = slice(i * step, (i + 1) * step)
            se = slice(i * step // 8, (i + 1) * step // 8)
            jobs.append(_pool().submit(_enc_bf9_chunk, x[s], qd[s], ed[se]))
    for j in jobs:
        j.result()


def _enc_bf9(x):
    """x f32 [..., n*8] -> (q uint8 bytes flat, e uint8 flat); serial ref."""
    q = np.empty(x.size, np.uint8)
    e = np.empty(x.size // 8, np.uint8)
    _enc_bf9_chunk(x.reshape(-1), q, e)
    return q, e


def _dec_bf9_host(qbytes, e, shape):
    q = (qbytes.astype(np.float32) - 128.0).reshape(-1, 8)   # excess-128 uint8
    f = np.ldexp(np.float32(1.0), e.astype(np.int32) - 133).astype(np.float32)
    return (q * f.reshape(-1, 1)).reshape(shape)


def _setup():
    nc = _build()
    devs = jax.devices()[:NC]
    mesh = Mesh(np.asarray(devs), ("core",))
    sh_core = NamedSharding(mesh, P("core"))

    # --- prep jit: decode bf9, all-gather x, transpose, RoPE tables ---
    inv_np = (THETA ** (-np.arange(0, DH, 2, dtype=np.float64) / DH)).astype(np.float32)

    def _dec(qu8, e, r, c):
        q = jax.lax.bitcast_convert_type(qu8.reshape(r, c), jnp.int8).astype(jnp.float32)
        f = jnp.exp2(e.reshape(r, c // 8).astype(jnp.float32) - 134.0)
        return q * jnp.repeat(f, 8, axis=1)

    # stage 1: pure bf9 decode (fused uint8 -> xl bf16, bl f32, pos f32)
    def _decode(fl):
        # fl [1, _L8] uint8: x|w mantissas, x|w exponents, position limbs
        m = fl[0, :_MQ]
        e = fl[0, _MQ:_MQ + _ME]
        limbs = fl[0, _MQ + _ME:].reshape(3, S).astype(jnp.float32)   # exact ints < 256
        pos = (limbs[0] * 65536.0 + limbs[1] * 256.0 + limbs[2]).reshape(1, S)
        full = _dec(m, e, (R // NC) + 128 * 4, D)                     # [1024, 1024] f32
        xl = full[:R // NC].astype(jnp.bfloat16)                      # [512, 1024]
        bl = full[R // NC:].reshape(128, 4 * D)                       # [128, 4096] f32
        return xl, bl, pos

    decode = jax.jit(shard_map(
        _decode, mesh=mesh, in_specs=(P("core"),),
        out_specs=(P("core"),) * 3, check_rep=False))

    # stage 2: all-gather x, transpose, RoPE tables, weight split
    def _prep(xl, bl, pl):
        pos = pl[0]                                                   # [2048] f32
        xfull = jax.lax.all_gather(xl, "core", axis=0, tiled=True)    # [4096, 1024]
        xt = xfull.T                                                  # [1024, 4096] bf16
        ang = inv_np[:, None] * pos[None, :]                          # [32, 2048]
        cos32 = jnp.cos(ang)
        sin32 = jnp.sin(ang)
        cost = jnp.concatenate([cos32] * 4, axis=0)                   # [128, S] f32
        sint = jnp.concatenate([-sin32, sin32, -sin32, sin32], axis=0).astype(jnp.bfloat16)
        wq = bl[:, 0 * D:1 * D].astype(jnp.bfloat16)
        wk = bl[:, 1 * D:2 * D].astype(jnp.bfloat16)
        wv = bl[:, 2 * D:3 * D].astype(jnp.bfloat16)
        wo = bl[:, 3 * D:4 * D]                                       # f32
        return xt, wq, wk, wv, wo, cost, sint

    prep = jax.jit(shard_map(
        _prep, mesh=mesh,
        in_specs=(P("core"),) * 3,
        out_specs=(P("core"),) * 7, check_rep=False))

    # --- bass jit (adapted from bass2jax.run_bass_via_pjrt, cached + no
    #     donated zero outputs) ---
    b2j.install_neuronx_cc_hook()
    partition_name = nc.partition_id_tensor.name if nc.partition_id_tensor else None
    in_names, out_names, out_avals = [], [], []
    for alloc in nc.m.functions[0].allocations:
        if not isinstance(alloc, mybir.MemoryLocationSet):
            continue
        name = alloc.memorylocations[0].name
        if alloc.kind == "ExternalInput":
            if name != partition_name:
                in_names.append(name)
        elif alloc.kind == "ExternalOutput":
            out_names.append(name)
            out_avals.append(jax.core.ShapedArray(
                tuple(alloc.tensor_shape), mybir.dt.np(alloc.dtype)))

    def _body(*args):
        operands = list(args)
        if partition_name is not None:
            operands.append(b2j.partition_id_tensor())
        outs = b2j._bass_exec_p.bind(
            *operands,
            out_avals=tuple(out_avals),
            in_names=tuple(in_names) + ((partition_name,) if partition_name else ()),
            out_names=tuple(out_names),
            lowering_input_output_aliases=(),
            sim_require_finite=True,
            sim_require_nnan=True,
            nc=nc,
        )
        return tuple(outs)

    bass_call = jax.jit(shard_map(
        _body, mesh=mesh,
        in_specs=(P("core"),) * len(in_names),
        out_specs=(P("core"),) * len(out_names), check_rep=False))

    # --- reduce jit: on-device 8-way partial sum + bf9 encode (4.5 MB).
    # Mantissas are stored excess-128 (q+128 as uint8): the device's
    # f32->int8 conversion saturates negatives to 0. ---
    def _reduce(yl):
        y = jax.lax.psum(yl.astype(jnp.float32), "core")              # [4096, 1024]
        r, c = y.shape
        bm = jnp.maximum(jnp.max(jnp.abs(y.reshape(r, c // 8, 8)), axis=2), 1e-8)
        b = (jax.lax.bitcast_convert_type(bm, jnp.int32) >> 23) & 0xFF
        factor = jnp.exp2(133.0 - b.astype(jnp.float32))
        q = jnp.clip(jnp.round(y * jnp.repeat(factor, 8, axis=1)) + 128.0,
                     1, 255).astype(jnp.uint8)
        return jnp.concatenate([q, b.astype(jnp.uint8)], axis=1)      # [4096, 1152]

    reduce = jax.jit(shard_map(
        _reduce, mesh=mesh, in_specs=(P("core"),), out_specs=P(),
        check_rep=False))

    # --- device-resident constants (one-time 1.7 MB upload) ---
    ones_c = jax.device_put(np.ones((NC * 128, DH), np.float32), sh_core)
    ident_c = jax.device_put(
        np.tile(np.eye(128, dtype=np.float32), (NC, 1)), sh_core)
    kk = np.arange(128)[:, None]
    qq = np.arange(256)[None, :]
    tri = np.concatenate([(qq >= kk).astype(npbf),
                          (qq - 128 >= kk).astype(npbf)], 1)          # [128, 512]
    trim_c = jax.device_put(np.tile(tri, (NC, 1)), sh_core)

    return dict(mesh=mesh, sh_core=sh_core, decode=decode, prep=prep,
                bass_call=bass_call, reduce=reduce, ones_c=ones_c,
                ident_c=ident_c, trim_c=trim_c, in_names=in_names)


def kernel(in_features, token_positions, Wq, Wk, Wv, Wo):
    if "ctx" not in _CACHE:
        _CACHE["ctx"] = _setup()
    C = _CACHE["ctx"]
    sh_core = C["sh_core"]

    # one fused upload [8, _L8] uint8 (~9 MB): x|w bf9 mantissas, exponents,
    # pos limbs. Encodes run thread-parallel straight into the fused rows.
    fused = np.empty((NC, _L8), dtype=np.uint8)
    xf = np.ascontiguousarray(in_features, dtype=np.float32).reshape(-1)
    blob = _weight_blob(np.asarray(Wq, np.float32), np.asarray(Wk, np.float32),
                        np.asarray(Wv, np.float32), np.asarray(Wo, np.float32))
    bf = blob.reshape(-1)
    jobs = []
    for c in range(NC):
        jobs.append((xf[c * _XQ:(c + 1) * _XQ], fused[c, :_XQ],
                     fused[c, _MQ:_MQ + _XQ // 8]))
        jobs.append((bf[c * _WQ:(c + 1) * _WQ], fused[c, _XQ:_MQ],
                     fused[c, _MQ + _XQ // 8:_MQ + _ME]))
    _enc_bf9_par(jobs, nchunk=1)
    p = np.asarray(token_positions, np.int64)
    limbs = np.stack([(p >> 16) & 255, (p >> 8) & 255, p & 255]).astype(np.uint8)
    fused[:, _MQ + _ME:] = limbs.reshape(1, _PN)

    fd = jax.device_put(fused, sh_core)       # single RPC

    xl, bl, pl = C["decode"](fd)
    xt, wq, wk, wv, wo, cost, sint = C["prep"](xl, bl, pl)
    (yo,) = C["bass_call"](xt, wq, wk, wv, wo, cost, sint,
                           C["ones_c"], C["ident_c"], C["trim_c"])
    packed = np.asarray(C["reduce"](yo))      # [4096, 1152] uint8, 4.5 MB
    y = _dec_bf9_host(np.ascontiguousarray(packed[:, :D]), packed[:, D:], (R, D))
    return y.reshape(B, S, D)


# revision 21
# speedup vs baseline: 1.1813x; 1.0671x over previous
"""Causal multi-head self-attention (B=2, S=2048, D=1024, H=16) on 8 trn2 cores.

Device kernel (unchanged from v1): tensor-parallel over heads — core c owns
heads (2c, 2c+1), both batches, full sequence. Per core: QKV projections for
its 2 heads, RoPE, causal attention, output-projection partial product into
yo [4096, 1024] bf16.

Dispatch layer: the wall-clock metric is dominated by the ~45 MB/s axon
tunnel (fixed ~150 ms/put, ~85 ms/dispatch round trip; chained jit
dispatches are free), so per-call host<->device traffic is cut from
~207 MB to 16 MB up + 4.5 MB down (4.4 s -> ~0.53 s):
  - ONE fused device_put [8, L] bf16, P('core')-sharded: per-core row =
    x row-shard (1 MB) | per-core weight slices wq|wk|wv|wo (1 MB) |
    token_positions as 3 base-256 bf16 limbs (exact for p < 2^24).
    Extra puts cost ~90 ms each in RPC serialization, hence ONE.
  - an XLA prep jit all-gathers x on device over NeuronLink, transposes
    to xt [D, R], splits the weight blob (wo cast back to f32), and
    builds the RoPE cos/sin tables on device (12 MB never crosses the
    tunnel).
  - the bass jit binds the prepped device arrays directly; no donated
    zero output buffers (the original path shipped 64 MB of zeros per
    call; yo is fully written by the kernel so uninit PJRT allocation
    is fine). ident/trim/ones are input-independent device constants.
  - the 8 partial yo's are psum-reduced ON DEVICE (f32 accumulate) and
    block-float9-encoded on device (int8 mantissa excess-128 + shared
    exponent per 8 els — the device f32->int8 convert saturates
    negatives, hence excess-128); one 4.5 MB uint8 replica is fetched
    and decoded on the host. Quantization adds ~4e-3 max-rel (total
    8.4e-3 vs the 2e-2 gate). bf9 on the UPLINK was tried and is a
    wash: the host is 1 CPU, so the numpy encode costs what the wire
    saves (kernel_v4.py).
  - all jits are built once and cached; the original run_bass_kernel_spmd
    path re-traced + re-lowered (incl. zstd of the BIR) every call.
"""
import math
import numpy as np

import jax
import jax.numpy as jnp
from jax.sharding import Mesh, PartitionSpec as P, NamedSharding

from jax.experimental.shard_map import shard_map

import ml_dtypes

import concourse.bass as bass
from concourse import bacc
import concourse.mybir as mybir
from concourse.tile import TileContext
from concourse import bass2jax as b2j

THETA = 10000.0
B, S, D, H = 2, 2048, 1024, 16
DH = D // H          # 64
NC = 8               # cores
HPC = H // NC        # heads per core = 2
R = B * S            # 4096 flat rows
SCALE = 1.0 / math.sqrt(DH)

f32 = mybir.dt.float32
f32r = mybir.dt.float32r
bf16 = mybir.dt.bfloat16
npbf = ml_dtypes.bfloat16

_CACHE = {}


def _build(_DBG_REPS=1):
    nc = bacc.Bacc(num_devices=NC)

    xt = nc.declare_dram_parameter("xt", [D, R], bf16, isOutput=False)
    wq = nc.declare_dram_parameter("wq", [128, 8 * 2 * DH], bf16, isOutput=False)
    wk = nc.declare_dram_parameter("wk", [128, 8 * 2 * DH], bf16, isOutput=False)
    wv = nc.declare_dram_parameter("wv", [128, 8 * 2 * DH], bf16, isOutput=False)
    wo = nc.declare_dram_parameter("wo", [2 * DH, D], f32r, isOutput=False)
    cost = nc.declare_dram_parameter("cost", [128, S], f32, isOutput=False)
    sint = nc.declare_dram_parameter("sint", [128, S], bf16, isOutput=False)
    ones = nc.declare_dram_parameter("ones", [128, DH], f32r, isOutput=False)
    ident = nc.declare_dram_parameter("ident", [128, 128], f32, isOutput=False)
    trim = nc.declare_dram_parameter("trim", [128, 512], bf16, isOutput=False)
    yo = nc.declare_dram_parameter("yo", [R, D], bf16, isOutput=True)

    NQ = 4            # xT column quarters in phase A
    QW = R // NQ      # 1024 rows per quarter
    NB = QW // 512    # 2 proj psum blocks per quarter
    KT = S // 128     # 16 key tiles per batch

    with TileContext(nc) as tc:
        import contextlib
        ctx = contextlib.ExitStack()
        with ctx:
            # ---- persistent pools (whole kernel) ----
            pers = ctx.enter_context(tc.tile_pool(name="pers", bufs=1))
            exp_pool = ctx.enter_context(tc.tile_pool(name="expp", bufs=8))
            aux = ctx.enter_context(tc.tile_pool(name="aux", bufs=2))

            q_rope = pers.tile([128, R], bf16, name="q_rope")
            k_rope = pers.tile([128, R], bf16, name="k_rope")
            # V per batch, natural rows layout; per key tile 130 cols:
            #   per head: [v(64) | one]
            v_sb = [pers.tile([128, KT * 130], bf16, name=f"v_sb{b}") for b in range(B)]
            attn = [pers.tile([128, S], f32r, name=f"attn{b}") for b in range(B)]
            wo_sb = pers.tile([128, D], f32r, name="wo_sb")
            ones_sb = pers.tile([128, DH], f32r, name="ones_sb")
            id_sb = pers.tile([128, 128], f32, name="id_sb")
            tri_sb = pers.tile([128, 512], bf16, name="tri_sb")
            for b in range(B):
                nc.vector.memset(
                    v_sb[b][:, :].rearrange("p (t c) -> p t c", c=65)[:, :, 64:65], 1.0)

            # ---- phase A: projections + RoPE + V assembly ----
            for _rep in range(_DBG_REPS):
              with tc.tile_pool(name="phA", bufs=1) as pha, \
                   tc.tile_pool(name="xtp", bufs=2) as xtp, \
                   tc.tile_pool(name="ropetmp", bufs=2) as rtp, \
                   tc.tile_pool(name="prj_ps", bufs=6, space="PSUM") as prj, \
                   tc.tile_pool(name="tp_ps", bufs=2, space="PSUM") as tpp:

                  wq_sb = pha.tile([128, 8, 2 * DH], bf16, name="wq_sb")
                  wk_sb = pha.tile([128, 8, 2 * DH], bf16, name="wk_sb")
                  wv_sb = pha.tile([128, 8, 2 * DH], bf16, name="wv_sb")
                  cos_sb = pha.tile([128, S], f32, name="cos_sb")
                  sin_sb = pha.tile([128, S], bf16, name="sin_sb")
                  nc.sync.dma_start(out=wq_sb[:, :, :], in_=wq.rearrange("p (t m) -> p t m", t=8))
                  xtb0 = [xtp.tile([128, 4, QW], bf16, name=f"xt0{hh}", tag=f"xt{hh}")
                          for hh in range(2)]
                  for hh in range(2):
                      for sub in range(2):
                          nc.sync.dma_start(
                              out=xtb0[hh][:, sub * 2:(sub + 1) * 2, :],
                              in_=xt[hh * 512 + sub * 256:hh * 512 + (sub + 1) * 256,
                                     0:QW].rearrange("(t p) m -> p t m", p=128))
                  nc.sync.dma_start(out=wk_sb[:, :, :], in_=wk.rearrange("p (t m) -> p t m", t=8))
                  nc.sync.dma_start(out=wv_sb[:, :, :], in_=wv.rearrange("p (t m) -> p t m", t=8))
                  nc.sync.dma_start(out=cos_sb[:, :], in_=cost[:, :])
                  nc.sync.dma_start(out=sin_sb[:, :], in_=sint[:, :])
                  if _rep == 0:
                      nc.sync.dma_start(out=id_sb[:, :], in_=ident[:, :])
                      nc.sync.dma_start(out=tri_sb[:, :], in_=trim[:, :])
                      nc.sync.dma_start(out=ones_sb[:, :], in_=ones[:, :])
                      nc.sync.dma_start(out=wo_sb[:, :], in_=wo[:, :])

                  for qr in range(NQ):
                      c0 = qr * QW           # global row offset of this quarter
                      s0 = c0 % S            # seq offset of this quarter
                      if qr == 0:
                          xtb = xtb0
                      else:
                          xtb = xtb_next
                      if qr + 1 < NQ:
                          cn = (qr + 1) * QW
                          xtb_next = [xtp.tile([128, 4, QW], bf16, name=f"xt{qr + 1}{hh}", tag=f"xt{hh}")
                                      for hh in range(2)]
                          for hh in range(2):
                              nc.sync.dma_start(
                                  out=xtb_next[hh][:, :, :],
                                  in_=xt[hh * 512:(hh + 1) * 512, cn:cn + QW].rearrange(
                                      "(t p) m -> p t m", p=128))
                      xts = [xtb[k // 4][:, k % 4, :] for k in range(8)]

                      for tname, wsb, rope in (("q", wq_sb, q_rope), ("k", wk_sb, k_rope)):
                          pss = []
                          for n in range(NB):
                              ps = prj.tile([128, 512], f32, name=f"p{tname}{qr}{n}", tag="prj")
                              for k in range(8):
                                  nc.tensor.matmul(ps[:, :], wsb[:, k, :], xts[k][:, n * 512:(n + 1) * 512],
                                                   start=(k == 0), stop=(k == 7))
                              pss.append(ps)
                          # RoPE: per head rows [even(32) | odd(32)]; sin table
                          # carries the sign, so out = ps*cos + swap32(ps)*sin.
                          raw = rtp.tile([128, QW], bf16, name=f"raw{tname}", tag="raw")
                          pcs = rtp.tile([128, QW], bf16, name=f"pcs{tname}", tag="pcs")
                          for n in range(NB):
                              cs = slice(n * 512, (n + 1) * 512)
                              gs = slice(s0 + n * 512, s0 + (n + 1) * 512)
                              nc.vector.tensor_copy(raw[:, cs], pss[n][:, :])
                              nc.vector.tensor_tensor(pcs[:, cs], pss[n][:, :], cos_sb[:, gs], mybir.AluOpType.mult)
                          swp = rtp.tile([128, QW], bf16, name=f"swp{tname}", tag="swp")
                          for r0 in range(0, 128, 64):
                              nc.sync.dma_start(out=swp[r0:r0 + 32, :], in_=raw[r0 + 32:r0 + 64, :])
                              nc.sync.dma_start(out=swp[r0 + 32:r0 + 64, :], in_=raw[r0:r0 + 32, :])
                          gq = slice(s0, s0 + QW)
                          tsn = rtp.tile([128, QW], bf16, name=f"tsn{tname}", tag="tsn")
                          nc.vector.tensor_tensor(tsn[:, :], swp[:, :], sin_sb[:, gq], mybir.AluOpType.mult)
                          nc.vector.tensor_tensor(rope[:, c0:c0 + QW], pcs[:, :], tsn[:, :], mybir.AluOpType.add)

                      # V: transposed projection (N=512) then PE-transpose to natural
                      vt_sb = rtp.tile([128, QW], f32, name=f"vt{qr}", tag="vt")
                      for n in range(NB):
                          ps = prj.tile([128, 512], f32, name=f"pv{qr}{n}", tag="prj")
                          for k in range(8):
                              nc.tensor.matmul(ps[:, :], wv_sb[:, k, :], xts[k][:, n * 512:(n + 1) * 512],
                                               start=(k == 0), stop=(k == 7))
                          nc.scalar.copy(vt_sb[:, n * 512:(n + 1) * 512], ps[:, :])
                      for rt in range(QW // 128):
                          gr = c0 + rt * 128                    # global row
                          sk = (gr % S) // 128                  # key tile within batch
                          vb = v_sb[gr // S]
                          tp = tpp.tile([128, 128], f32, name=f"tp{qr}{rt}", tag="tp")
                          nc.tensor.transpose(tp[:, :], vt_sb[:, rt * 128:(rt + 1) * 128], id_sb[:, :])
                          dst = vb[:, sk * 130: sk * 130 + 130].rearrange("p (h c) -> p h c", c=65)
                          nc.vector.tensor_copy(dst[:, :, 0:64],
                                                tp[:, :].rearrange("p (h c) -> p h c", c=64))

              # ---- attention + output projection ----
              rctx = contextlib.ExitStack()
              sc_ps = rctx.enter_context(tc.tile_pool(name=f"sc_ps{_rep}", bufs=2, space="PSUM"))
              pv_ps = rctx.enter_context(tc.tile_pool(name=f"pv_ps{_rep}", bufs=2, space="PSUM"))
              ax_ps = rctx.enter_context(tc.tile_pool(name=f"ax_ps{_rep}", bufs=2, space="PSUM"))
              ysb_pool = rctx.enter_context(tc.tile_pool(name=f"ysb{_rep}", bufs=2))
              def emit_outproj(pb, pqh, py_sb, rts):
                  pcc = pb * 2 + pqh
                  for rt in rts:
                      gr = pb * S + pqh * 1024 + rt * 128
                      for nb2 in range(2):
                          yp = ax_ps.tile([128, 512], f32, name=f"yp{_rep}{pcc}{rt}{nb2}", tag="axp")
                          nc.tensor.matmul(yp[:, :], attn[pb][:, (gr % S):(gr % S) + 128],
                                           wo_sb[:, nb2 * 512:(nb2 + 1) * 512],
                                           start=True, stop=True)
                          nc.vector.tensor_copy(py_sb[:, rt, nb2 * 512:(nb2 + 1) * 512], yp[:, :])

              def emit_ydma(pb, pqh, py_sb, fine=False):
                  r0 = pb * S + pqh * 1024
                  n = 8 if fine else 4
                  w = 8 // n
                  for hh in range(n):
                      rr = r0 + hh * 128 * w
                      nc.sync.dma_start(
                          out=yo[rr:rr + 128 * w, :].rearrange("(t p) m -> p t m", p=128),
                          in_=py_sb[:, hh * w:(hh + 1) * w, :])

              pending = None   # previous chunk's outproj, interleaved into this one
              for b in range(B):
                  for qh in range(2):              # row-chunk of 1024 (4 q-blocks)
                      cc = b * 2 + qh
                      y_sb = ysb_pool.tile([128, 8, D], bf16, name=f"ysb{_rep}{cc}", tag="ysb")
                      group_idx = 0
                      for h in (1, 0):             # h=1 first: its attn DMA overlaps
                          for qp in range(2):      # h=0 compute
                              qbs = (qh * 4 + qp * 2, qh * 4 + qp * 2 + 1)
                              pvt = pv_ps.tile([65, 512], f32, name=f"pv{_rep}{cc}{h}{qp}", tag="pv")
                              pv = pvt[0:65, :]
                              for qi, qb in enumerate(qbs):
                                  nsk = 2 * (qb + 1)
                                  q_sl = slice(b * S + qb * 256, b * S + (qb + 1) * 256)
                                  # 2nd diagonal tile (sk=2qb+1): its first 128 q
                                  # cols are fully masked — compute only the valid
                                  # half (qb>=1; qb=0 keeps the plain path)
                                  for ch0 in range(0, nsk, 4):
                                      m = min(4, nsk - ch0)
                                      last_chunk = (ch0 + m == nsk)
                                      trim = qb >= 1 and last_chunk
                                      sc = sc_ps.tile([128, 1024], f32, name=f"sc{_rep}{cc}{h}{qp}{qi}{ch0}", tag="sc")
                                      for j in range(m):
                                          sk = ch0 + j
                                          k_sl = slice(b * S + sk * 128, b * S + (sk + 1) * 128)
                                          if trim and sk == 2 * qb + 1:
                                              qh_sl = slice(b * S + qb * 256 + 128,
                                                            b * S + (qb + 1) * 256)
                                              nc.tensor.matmul(sc[:, j * 256:j * 256 + 128],
                                                               k_rope[64 * h:64 * h + 64, k_sl],
                                                               q_rope[64 * h:64 * h + 64, qh_sl],
                                                               start=True, stop=True)
                                          else:
                                              o = slice(j * 256, (j + 1) * 256)
                                              nc.tensor.matmul(sc[:, o], k_rope[64 * h:64 * h + 64, k_sl],
                                                               q_rope[64 * h:64 * h + 64, q_sl],
                                                               start=True, stop=True)
                                      ncols = (m - 1) * 256 + 128 if trim else m * 256
                                      ex = exp_pool.tile([128, 1024], bf16, name=f"ex{_rep}{cc}{h}{qp}{qi}{ch0}", tag="ex")
                                      nc.scalar.activation(ex[:, 0:ncols], sc[:, 0:ncols],
                                                           mybir.ActivationFunctionType.Exp, scale=SCALE)
                                      for j in range(m):
                                          sk = ch0 + j
                                          o = slice(j * 256, (j + 1) * 256)
                                          if sk == 2 * qb:      # diagonal masking
                                              nc.vector.tensor_tensor(ex[:, o], ex[:, o], tri_sb[:, 0:256],
                                                                      mybir.AluOpType.mult)
                                          elif sk == 2 * qb + 1:
                                              if trim:
                                                  oh = slice(j * 256, j * 256 + 128)
                                                  nc.vector.tensor_tensor(ex[:, oh], ex[:, oh],
                                                                          tri_sb[:, 0:128],
                                                                          mybir.AluOpType.mult)
                                              else:
                                                  nc.vector.tensor_tensor(ex[:, o], ex[:, o], tri_sb[:, 256:512],
                                                                          mybir.AluOpType.mult)
                                      # PV: non-diag tiles, then the half-width 2nd
                                      # diagonal, then the full-width 1st diagonal
                                      # carrying the stop flag (accumulation is
                                      # commutative; stop must land on a matmul
                                      # covering every accumulated column)
                                      if trim:
                                          js = ([j for j in range(m) if ch0 + j < 2 * qb]
                                                + [m - 1, m - 2])
                                      else:
                                          js = ([j for j in range(m) if ch0 + j < 2 * qb]
                                                + [j for j in range(m) if ch0 + j >= 2 * qb])
                                      for j in js:
                                          sk = ch0 + j
                                          if trim and sk == 2 * qb + 1:
                                              nc.tensor.matmul(pv[:, qi * 256 + 128:(qi + 1) * 256],
                                                               v_sb[b][:, sk * 130 + 65 * h: sk * 130 + 65 * h + 65],
                                                               ex[:, j * 256:j * 256 + 128],
                                                               start=False, stop=False)
                                          else:
                                              o = slice(j * 256, (j + 1) * 256)
                                              stop = (sk == 2 * qb) if trim else (sk == nsk - 1)
                                              nc.tensor.matmul(pv[:, qi * 256:(qi + 1) * 256],
                                                               v_sb[b][:, sk * 130 + 65 * h: sk * 130 + 65 * h + 65],
                                                               ex[:, o],
                                                               start=(sk == 0), stop=stop)
                              # normalize: attn rows = pv_vals * (1/rowsum broadcast)
                              rec = aux.tile([1, 512], f32r, name=f"rec{_rep}{cc}{h}{qp}", tag="rec")
                              with nc.allow_low_precision(reason="softmax reciprocal"):
                                  nc.vector.reciprocal(rec[0:1, :], pv[64:65, :])
                              bc = aux.tile([64, 512], f32r, name=f"bc{_rep}{cc}{h}{qp}", tag="bc")
                              nc.gpsimd.partition_broadcast(bc[0:64, :], rec[0:1, :], channels=64)
                              a_sl = slice((qh * 2 + qp) * 512, (qh * 2 + qp + 1) * 512)
                              if h == 0:
                                  nc.vector.tensor_tensor(attn[b][0:64, a_sl], pv[0:64, :], bc[0:64, :],
                                                          mybir.AluOpType.mult)
                              else:
                                  hs_q = aux.tile([64, 512], f32r, name=f"hs{_rep}{cc}{qp}", tag="hs")
                                  nc.vector.tensor_tensor(hs_q[0:64, :], pv[0:64, :], bc[0:64, :],
                                                          mybir.AluOpType.mult)
                                  nc.sync.dma_start(out=attn[b][64:128, a_sl], in_=hs_q[0:64, :])
                              # interleave previous chunk's output projection into
                              # this chunk's Act-bound stalls (2 of its 8 row-tiles
                              # per group)
                              if pending is not None:
                                  pb, pqh, py_sb = pending
                                  emit_outproj(pb, pqh, py_sb,
                                               range(group_idx * 2, group_idx * 2 + 2))
                              group_idx += 1
                      if pending is not None:
                          emit_ydma(*pending)
                      pending = (b, qh, y_sb)
              # flush the last chunk's output projection
              emit_outproj(*pending, rts=range(8))
              emit_ydma(*pending, fine=True)
              rctx.close()
    nc.finalize()
    return nc


# ---------------------------------------------------------------------------
# host-side weight prelayout (cheap numpy; ~8 MB bf16 total)
# ---------------------------------------------------------------------------

def _weight_blob(Wq, Wk, Wv, Wo):
    """Per-core [128, 4096] bf16: [wq | wk | wv | wo] columns; stacked to
    [1024, 4096] for P('core') sharding."""
    blob = np.empty((NC * 128, 4 * D), dtype=npbf)
    for c in range(NC):
        h0 = HPC * c
        rows = []
        for j in range(HPC):
            rows += [(h0 + j) * DH + 2 * i for i in range(DH // 2)]      # evens
            rows += [(h0 + j) * DH + 2 * i + 1 for i in range(DH // 2)]  # odds

        def _prelayout(w):                                               # [D,128] -> [128, 8*128]
            return w.reshape(8, 128, 128).transpose(1, 0, 2).reshape(128, 8 * 128)

        r0 = c * 128
        blob[r0:r0 + 128, 0 * D:1 * D] = _prelayout(Wq[rows, :].T).astype(npbf)
        blob[r0:r0 + 128, 1 * D:2 * D] = _prelayout(Wk[rows, :].T).astype(npbf)
        vrows = list(range(h0 * DH, (h0 + HPC) * DH))
        blob[r0:r0 + 128, 2 * D:3 * D] = _prelayout(Wv[vrows, :].T).astype(npbf)
        blob[r0:r0 + 128, 3 * D:4 * D] = Wo[:, vrows].T.astype(npbf)     # [128, D]
    return blob


# ---------------------------------------------------------------------------
# cached jits
# ---------------------------------------------------------------------------

_XN = (R // NC) * D          # 524288 x elements per core
_BN = 128 * 4 * D            # 524288 blob elements per core
_PN = 3 * S                  # 6144 position-limb elements per core
_LN = _XN + _BN + _PN        # fused row length per core


def _setup():
    nc = _build()
    devs = jax.devices()[:NC]
    mesh = Mesh(np.asarray(devs), ("core",))
    sh_core = NamedSharding(mesh, P("core"))

    # --- prep jit: all-gather x, transpose, RoPE tables, weight split ---
    inv_np = (THETA ** (-np.arange(0, DH, 2, dtype=np.float64) / DH)).astype(np.float32)

    def _prep(fl):
        # fl [1, _LN] bf16: x row-shard | weight blob | position limbs
        xl = fl[0, :_XN].reshape(R // NC, D)                          # [512, 1024]
        bl = fl[0, _XN:_XN + _BN].reshape(128, 4 * D)                 # [128, 4096]
        limbs = fl[0, _XN + _BN:].reshape(3, S).astype(jnp.float32)   # exact ints < 256
        pos = limbs[0] * 65536.0 + limbs[1] * 256.0 + limbs[2]        # [2048]
        xfull = jax.lax.all_gather(xl, "core", axis=0, tiled=True)    # [4096, 1024]
        xt = xfull.T                                                  # [1024, 4096] bf16
        ang = inv_np[:, None] * pos[None, :]                          # [32, 2048]
        cos32 = jnp.cos(ang)
        sin32 = jnp.sin(ang)
        cost = jnp.concatenate([cos32] * 4, axis=0)                   # [128, S] f32
        sint = jnp.concatenate([-sin32, sin32, -sin32, sin32], axis=0).astype(jnp.bfloat16)
        wq = bl[:, 0 * D:1 * D]
        wk = bl[:, 1 * D:2 * D]
        wv = bl[:, 2 * D:3 * D]
        wo = bl[:, 3 * D:4 * D].astype(jnp.float32)
        return xt, wq, wk, wv, wo, cost, sint

    prep = jax.jit(shard_map(
        _prep, mesh=mesh,
        in_specs=(P("core"),),
        out_specs=(P("core"),) * 7, check_rep=False))

    # --- bass jit (adapted from bass2jax.run_bass_via_pjrt, cached + no
    #     donated zero outputs) ---
    b2j.install_neuronx_cc_hook()
    partition_name = nc.partition_id_tensor.name if nc.partition_id_tensor else None
    in_names, out_names, out_avals = [], [], []
    for alloc in nc.m.functions[0].allocations:
        if not isinstance(alloc, mybir.MemoryLocationSet):
            continue
        name = alloc.memorylocations[0].name
        if alloc.kind == "ExternalInput":
            if name != partition_name:
                in_names.append(name)
        elif alloc.kind == "ExternalOutput":
            out_names.append(name)
            out_avals.append(jax.core.ShapedArray(
                tuple(alloc.tensor_shape), mybir.dt.np(alloc.dtype)))

    def _body(*args):
        operands = list(args)
        if partition_name is not None:
            operands.append(b2j.partition_id_tensor())
        outs = b2j._bass_exec_p.bind(
            *operands,
            out_avals=tuple(out_avals),
            in_names=tuple(in_names) + ((partition_name,) if partition_name else ()),
            out_names=tuple(out_names),
            lowering_input_output_aliases=(),
            sim_require_finite=True,
            sim_require_nnan=True,
            nc=nc,
        )
        return tuple(outs)

    bass_call = jax.jit(shard_map(
        _body, mesh=mesh,
        in_specs=(P("core"),) * len(in_names),
        out_specs=(P("core"),) * len(out_names), check_rep=False))

    # --- reduce jit: on-device 8-way partial sum + bf9 encode (4.5 MB).
    # Mantissas stored excess-128 (device f32->int8 saturates negatives). ---
    def _reduce(yl):
        y = jax.lax.psum(yl.astype(jnp.float32), "core")              # [4096, 1024]
        r, c = y.shape
        bm = jnp.maximum(jnp.max(jnp.abs(y.reshape(r, c // 8, 8)), axis=2), 1e-8)
        b = (jax.lax.bitcast_convert_type(bm, jnp.int32) >> 23) & 0xFF
        factor = jnp.exp2(133.0 - b.astype(jnp.float32))
        q = jnp.clip(jnp.round(y * jnp.repeat(factor, 8, axis=1)) + 128.0,
                     1, 255).astype(jnp.uint8)
        return jnp.concatenate([q, b.astype(jnp.uint8)], axis=1)      # [4096, 1152]

    reduce = jax.jit(shard_map(
        _reduce, mesh=mesh, in_specs=(P("core"),), out_specs=P(),
        check_rep=False))

    # --- device-resident constants (one-time 1.7 MB upload) ---
    ones_c = jax.device_put(np.ones((NC * 128, DH), np.float32), sh_core)
    ident_c = jax.device_put(
        np.tile(np.eye(128, dtype=np.float32), (NC, 1)), sh_core)
    kk = np.arange(128)[:, None]
    qq = np.arange(256)[None, :]
    tri = np.concatenate([(qq >= kk).astype(npbf),
                          (qq - 128 >= kk).astype(npbf)], 1)          # [128, 512]
    trim_c = jax.device_put(np.tile(tri, (NC, 1)), sh_core)

    return dict(mesh=mesh, sh_core=sh_core, prep=prep, bass_call=bass_call,
                reduce=reduce, ones_c=ones_c, ident_c=ident_c, trim_c=trim_c,
                in_names=in_names)


def kernel(in_features, token_positions, Wq, Wk, Wv, Wo):
    if "ctx" not in _CACHE:
        _CACHE["ctx"] = _setup()
    C = _CACHE["ctx"]
    sh_core = C["sh_core"]

    # one fused upload [8, _LN] bf16: x row-shard | weight blob | pos limbs
    fused = np.empty((NC, _LN), dtype=npbf)
    x_bf = np.asarray(in_features, dtype=np.float32).reshape(NC, _XN)
    fused[:, :_XN] = x_bf.astype(npbf)
    blob = _weight_blob(np.asarray(Wq, np.float32), np.asarray(Wk, np.float32),
                        np.asarray(Wv, np.float32), np.asarray(Wo, np.float32))
    fused[:, _XN:_XN + _BN] = blob.reshape(NC, _BN)
    p = np.asarray(token_positions, np.int64)
    limbs = np.stack([(p >> 16) & 255, (p >> 8) & 255, p & 255]).astype(npbf)
    fused[:, _XN + _BN:] = limbs.reshape(1, _PN)

    fd = jax.device_put(fused, sh_core)       # 16 MB, single RPC

    xt, wq, wk, wv, wo, cost, sint = C["prep"](fd)
    (yo,) = C["bass_call"](xt, wq, wk, wv, wo, cost, sint,
                           C["ones_c"], C["ident_c"], C["trim_c"])
    packed = np.asarray(C["reduce"](yo))      # [4096, 1152] uint8, 4.5 MB
    q = (packed[:, :D].astype(np.float32) - 128.0).reshape(-1, 8)
    f = np.ldexp(np.float32(1.0), packed[:, D:].astype(np.int32) - 133)
    return (q * f.reshape(-1, 1).astype(np.float32)).reshape(B, S, D)
